# revision 6
# baseline (speedup 1.0000x reference)
"""Trainium2 Bass kernel for the 3-layer MLP encode/decode forward pass.

Computation (B = 65536):
    d_i = pinv(W_i)                       (host, negligible)
    h = lrelu(x @ W1.T)                   [B, 128]
    h = lrelu(h @ W2.T)                   [B, 64]
    h = h @ W3.T                          [B, 16]
    h = lrelu(h @ d3.T)                   [B, 64]   (folded: lrelu((d3@W3) @ h2))
    h = lrelu(h @ d2.T)                   [B, 128]
    out = h @ d1.T                        [B, 784]

Sharding: pure data-parallel — 8 cores x 8192 batch rows each; the tiny
weights (and host-side pinv) are replicated.

Per-core layout: activations are kept feature-major ([feat, batch]) so
TensorE contracts over features.  x is transposed on-chip via PE
transpose-mode (exact fp32).  The final layer swaps operand roles
(stationary = activation tile, moving = d1.T) so the output lands
batch-major in PSUM — no output transpose.  Matmuls run as float32r
(full PE rate at moving-N >= 256, ~tf32 rounding).

DMA: one 1.6MB transfer per 512-row tile each way ([128, 4*784] with 4
batch rows per partition — batch order inside a tile is permuted, which
cancels between the input transposes and the output writeback).
"""

import numpy as np

B = 65536
N_CORES = 8
B_LOC = B // N_CORES  # 8192
D0, D1, D2, D3 = 784, 128, 64, 16
KCH = 112          # 784 = 7 * 112 contraction chunks for layer 1
NKC = D0 // KCH    # 7
TILE = 512         # moving free dim per matmul (one fp32 PSUM bank)
SUB = 128          # batch sub-tile (partition dim of x / out tiles)
NSUB = TILE // SUB  # 4
HALF = D0 // 2     # 392


def _build_nc(b_loc=B_LOC, mm_dt_name="float32r", last_dt_name="float32r",
              act_name="Lrelu", repeat=1):
    import contextlib
    import concourse.tile as tile
    from concourse import bacc, mybir

    mm_dt = getattr(mybir.dt, mm_dt_name)
    last_dt = getattr(mybir.dt, last_dt_name)
    f32 = mybir.dt.float32
    LRELU = getattr(mybir.ActivationFunctionType, act_name)
    COPY = mybir.ActivationFunctionType.Copy

    nc = bacc.Bacc(trn_type="TRN2", target_bir_lowering=False, debug=False,
                   num_devices=N_CORES)

    x = nc.declare_dram_parameter("x", [b_loc, D0], f32, isOutput=False).ap()
    w1t = nc.declare_dram_parameter("w1t", [D0, D1], mm_dt, isOutput=False).ap()
    w2t = nc.declare_dram_parameter("w2t", [D1, D2], mm_dt, isOutput=False).ap()
    m3t = nc.declare_dram_parameter("m3t", [D2, D2], mm_dt, isOutput=False).ap()
    d2t = nc.declare_dram_parameter("d2t", [D2, D1], mm_dt, isOutput=False).ap()
    d1t = nc.declare_dram_parameter("d1t", [D1, D0], last_dt, isOutput=False).ap()
    ident = nc.declare_dram_parameter("ident", [SUB, SUB], f32, isOutput=False).ap()
    out = nc.declare_dram_parameter("out", [b_loc, D0], f32, isOutput=True).ap()

    n_tiles = b_loc // TILE
    # row = tile*512 + p*4 + s  (4 rows per partition -> one 1.6MB DMA per tile)
    x_r = x.rearrange("(n p s) f -> n p (s f)", p=SUB, s=NSUB)
    out_r = out.rearrange("(n p s) f -> n p (s f)", p=SUB, s=NSUB)

    with tile.TileContext(nc, num_cores=N_CORES) as tc:
        with (
            tc.tile_pool(name="consts", bufs=1) as consts,
            tc.tile_pool(name="xin", bufs=3) as xin,
            tc.tile_pool(name="xtp", bufs=14) as xtp,
            tc.tile_pool(name="acts", bufs=2) as acts,
            tc.tile_pool(name="outp", bufs=3) as outp,
            tc.tile_pool(name="psT", bufs=2, space="PSUM") as psT,
            tc.tile_pool(name="psMM", bufs=2, space="PSUM") as psMM,
            tc.tile_pool(name="psO", bufs=2, space="PSUM") as psO,
        ):
            # --- constants ---
            w1t_sb = consts.tile([KCH, NKC, D1], mm_dt)
            nc.sync.dma_start(out=w1t_sb, in_=w1t.rearrange("(c p) m -> p c m", p=KCH))
            w2t_sb = consts.tile([D1, D2], mm_dt)
            nc.sync.dma_start(out=w2t_sb, in_=w2t)
            m3t_sb = consts.tile([D2, D2], mm_dt)
            nc.sync.dma_start(out=m3t_sb, in_=m3t)
            d2t_sb = consts.tile([D2, D1], mm_dt)
            nc.sync.dma_start(out=d2t_sb, in_=d2t)
            d1t_sb = consts.tile([D1, D0], last_dt)
            nc.sync.dma_start(out=d1t_sb, in_=d1t)
            id_sb = consts.tile([SUB, SUB], f32)
            nc.sync.dma_start(out=id_sb, in_=ident)

            rep_ctx = (tc.For_i(0, repeat, 1) if repeat > 1
                       else contextlib.nullcontext())
            with rep_ctx:
              for t in range(n_tiles):
                # --- load 512 rows in one DMA: [128, 4, 784] ---
                x_sb = xin.tile([SUB, NSUB, D0], f32, tag="x")
                nc.sync.dma_start(out=x_sb, in_=x_r[t])

                # --- PE-transpose to feature-major: 7 chunks of [112, 512] ---
                xt_sb = []
                for c in range(NKC):
                    tp = psT.tile([KCH, TILE], f32, tag="psT")
                    for s in range(NSUB):
                        nc.tensor.transpose(
                            out=tp[:, s * SUB:(s + 1) * SUB],
                            in_=x_sb[:, s, c * KCH:(c + 1) * KCH],
                            identity=id_sb,
                        )
                    xt = xtp.tile([KCH, TILE], mm_dt, tag="xt")
                    nc.vector.tensor_copy(xt, tp)
                    xt_sb.append(xt)

                # --- L1: h1 = lrelu(W1 @ xT)  [128, 512] ---
                h1_ps = psMM.tile([D1, TILE], f32, tag="mm")
                for c in range(NKC):
                    nc.tensor.matmul(h1_ps, lhsT=w1t_sb[:, c, :], rhs=xt_sb[c],
                                     start=(c == 0), stop=(c == NKC - 1))
                h1_sb = acts.tile([D1, TILE], mm_dt, tag="h1")
                nc.scalar.activation(out=h1_sb, in_=h1_ps, func=LRELU, alpha=0.01)

                # --- L2: h2 = lrelu(W2 @ h1)  [64, 512] ---
                h2_ps = psMM.tile([D2, TILE], f32, tag="mm")
                nc.tensor.matmul(h2_ps, lhsT=w2t_sb, rhs=h1_sb,
                                 start=True, stop=True)
                h2_sb = acts.tile([D2, TILE], mm_dt, tag="h2")
                nc.scalar.activation(out=h2_sb, in_=h2_ps, func=LRELU, alpha=0.01)

                # --- L3 folded: g3 = lrelu((d3 @ W3) @ h2)  [64, 512] ---
                g3_ps = psMM.tile([D2, TILE], f32, tag="mm")
                nc.tensor.matmul(g3_ps, lhsT=m3t_sb, rhs=h2_sb,
                                 start=True, stop=True)
                g3_sb = acts.tile([D2, TILE], mm_dt, tag="g3")
                nc.scalar.activation(out=g3_sb, in_=g3_ps, func=LRELU, alpha=0.01)

                # --- L4: g2 = lrelu(d2 @ g3)  [128, 512] ---
                g2_ps = psMM.tile([D1, TILE], f32, tag="mm")
                nc.tensor.matmul(g2_ps, lhsT=d2t_sb, rhs=g3_sb,
                                 start=True, stop=True)
                g2_sb = acts.tile([D1, TILE], last_dt, tag="g2")
                nc.scalar.activation(out=g2_sb, in_=g2_ps, func=LRELU, alpha=0.01)

                # --- L5: out = g2.T @ d1.T, batch-major via stationary swap.
                # Two matmuls into one 2-bank PSUM tile ([:, :392] in bank 0,
                # [:, 512:904] in bank 1), one strided ACT copy out. ---
                o_sb = outp.tile([SUB, NSUB, D0], f32, tag="o")
                for s in range(NSUB):
                    g2c = g2_sb[:, s * SUB:(s + 1) * SUB]
                    po = psO.tile([SUB, 1024], f32, tag="po")
                    nc.tensor.matmul(po[:, :HALF], lhsT=g2c, rhs=d1t_sb[:, :HALF],
                                     start=True, stop=True)
                    nc.tensor.matmul(po[:, 512:512 + HALF], lhsT=g2c,
                                     rhs=d1t_sb[:, HALF:], start=True, stop=True)
                    po_v = po.rearrange("p (b r) -> p b r", b=2)[:, :, :HALF]
                    o_v = o_sb[:, s, :].rearrange("p (b r) -> p b r", b=2)
                    nc.scalar.activation(out=o_v, in_=po_v, func=COPY)
                nc.sync.dma_start(out=out_r[t], in_=o_sb)

    nc.finalize()
    return nc


def _host_weights(W1, W2, W3):
    def pinv(W):
        u, s, vh = np.linalg.svd(W.astype(np.float64), full_matrices=False)
        return (vh.T * (1.0 / s)) @ u.T

    d1, d2, d3 = pinv(W1), pinv(W2), pinv(W3)
    f = np.float32
    return {
        "w1t": np.ascontiguousarray(W1.T, dtype=f),
        "w2t": np.ascontiguousarray(W2.T, dtype=f),
        "m3t": np.ascontiguousarray((d3 @ W3.astype(np.float64)).T, dtype=f),
        "d2t": np.ascontiguousarray(d2.T, dtype=f),
        "d1t": np.ascontiguousarray(d1.T, dtype=f),
        "ident": np.eye(SUB, dtype=f),
    }


_NC_CACHE = {}


def _get_nc(key=("float32r", "float32r")):
    if key not in _NC_CACHE:
        _NC_CACHE[key] = _build_nc(B_LOC, key[0], key[1])
    return _NC_CACHE[key]


def kernel(x, W1, W2, W3):
    from concourse.bass_utils import run_bass_kernel_spmd

    x = np.ascontiguousarray(x, dtype=np.float32)
    w = _host_weights(np.asarray(W1), np.asarray(W2), np.asarray(W3))
    nc = _get_nc()
    in_maps = [
        {"x": x[i * B_LOC:(i + 1) * B_LOC], **w} for i in range(N_CORES)
    ]
    res = run_bass_kernel_spmd(nc, in_maps, core_ids=list(range(N_CORES)))
    return np.concatenate([res.results[i]["out"] for i in range(N_CORES)], axis=0)


# revision 13
# speedup vs baseline: 157.7926x; 157.7926x over previous
"""Trainium2 Bass kernel for the 3-layer MLP encode/decode forward pass.

Computation (B = 65536):
    d_i = pinv(W_i)                       (host, negligible)
    h = lrelu(x @ W1.T)                   [B, 128]
    h = lrelu(h @ W2.T)                   [B, 64]
    h = h @ W3.T                          [B, 16]
    h = lrelu(h @ d3.T)                   [B, 64]   (folded: lrelu((d3@W3) @ h2))
    h = lrelu(h @ d2.T)                   [B, 128]
    out = h @ d1.T                        [B, 784]

Sharding: pure data-parallel — 8 cores x 8192 batch rows each; the tiny
weights (and host-side pinv) are replicated.

Per-core layout: activations are kept feature-major ([feat, batch]) so
TensorE contracts over features.  x is transposed on-chip via PE
transpose-mode (exact fp32).  The final layer swaps operand roles
(stationary = activation tile, moving = d1.T) so the output lands
batch-major in PSUM — no output transpose.  Matmuls run as float32r
(full PE rate at moving-N >= 256, ~tf32 rounding).

DMA: one 1.6MB transfer per 512-row tile each way ([128, 4*784] with 4
batch rows per partition — batch order inside a tile is permuted, which
cancels between the input transposes and the output writeback).
"""

import numpy as np

B = 65536
N_CORES = 8
B_LOC = B // N_CORES  # 8192
D0, D1, D2, D3 = 784, 128, 64, 16
KCH = 112          # 784 = 7 * 112 contraction chunks for layer 1
NKC = D0 // KCH    # 7
TILE = 512         # moving free dim per matmul (one fp32 PSUM bank)
SUB = 128          # batch sub-tile (partition dim of x / out tiles)
NSUB = TILE // SUB  # 4
HALF = D0 // 2     # 392


def _build_nc(b_loc=B_LOC, mm_dt_name="float32r", last_dt_name="float32r",
              act_name="Lrelu", repeat=1, r_xpose=False, split_ocopy=False,
              bf16_out=False, xt_on_act=False, xin_bufs=3, outp_bufs=3,
              xtp_bufs=14, acts_bufs=2, out_dma_eng="sync", in_dma_eng="sync",
              alloc_mode="stack", staggered=False, no_l5=False):
    import contextlib
    import concourse.tile as tile
    from concourse import bacc, mybir

    mm_dt = getattr(mybir.dt, mm_dt_name)
    last_dt = getattr(mybir.dt, last_dt_name)
    f32 = mybir.dt.float32
    LRELU = getattr(mybir.ActivationFunctionType, act_name)
    COPY = mybir.ActivationFunctionType.Copy

    nc = bacc.Bacc(trn_type="TRN2", target_bir_lowering=False, debug=False,
                   num_devices=N_CORES)

    x = nc.declare_dram_parameter("x", [b_loc, D0], f32, isOutput=False).ap()
    w1t = nc.declare_dram_parameter("w1t", [D0, D1], mm_dt, isOutput=False).ap()
    w2t = nc.declare_dram_parameter("w2t", [D1, D2], mm_dt, isOutput=False).ap()
    m3t = nc.declare_dram_parameter("m3t", [D2, D2], mm_dt, isOutput=False).ap()
    d2t = nc.declare_dram_parameter("d2t", [D2, D1], mm_dt, isOutput=False).ap()
    d1t = nc.declare_dram_parameter("d1t", [D1, D0], last_dt, isOutput=False).ap()
    ident = nc.declare_dram_parameter("ident", [SUB, SUB], f32, isOutput=False).ap()
    out_dt = mybir.dt.bfloat16 if bf16_out else f32
    out = nc.declare_dram_parameter("out", [b_loc, D0], out_dt, isOutput=True).ap()

    n_tiles = b_loc // TILE
    # row = tile*512 + p*4 + s  (4 rows per partition -> one 1.6MB DMA per tile)
    x_r = x.rearrange("(n p s) f -> n p (s f)", p=SUB, s=NSUB)
    out_r = out.rearrange("(n p s) f -> n p (s f)", p=SUB, s=NSUB)

    with tile.TileContext(nc, num_cores=N_CORES, pool_alloc_mode=alloc_mode) as tc:
        with (
            tc.tile_pool(name="consts", bufs=1) as consts,
            tc.tile_pool(name="xin", bufs=xin_bufs) as xin,
            tc.tile_pool(name="xtp", bufs=xtp_bufs) as xtp,
            tc.tile_pool(name="acts", bufs=acts_bufs) as acts,
            tc.tile_pool(name="outp", bufs=outp_bufs) as outp,
            tc.tile_pool(name="psT", bufs=2, space="PSUM") as psT,
            tc.tile_pool(name="psMM", bufs=2, space="PSUM") as psMM,
            tc.tile_pool(name="psO", bufs=2, space="PSUM") as psO,
        ):
            # --- constants ---
            w1t_sb = consts.tile([KCH, NKC, D1], mm_dt)
            nc.sync.dma_start(out=w1t_sb, in_=w1t.rearrange("(c p) m -> p c m", p=KCH))
            w2t_sb = consts.tile([D1, D2], mm_dt)
            nc.sync.dma_start(out=w2t_sb, in_=w2t)
            m3t_sb = consts.tile([D2, D2], mm_dt)
            nc.sync.dma_start(out=m3t_sb, in_=m3t)
            d2t_sb = consts.tile([D2, D1], mm_dt)
            nc.sync.dma_start(out=d2t_sb, in_=d2t)
            d1t_sb = consts.tile([D1, D0], last_dt)
            nc.sync.dma_start(out=d1t_sb, in_=d1t)
            id_sb = consts.tile([SUB, SUB], f32)
            nc.sync.dma_start(out=id_sb, in_=ident)
            id_r = id_sb.bitcast(mybir.dt.float32r)

            rep_ctx = (tc.For_i(0, repeat, 1, staggered_reset=staggered)
                       if repeat > 1 else contextlib.nullcontext())
            with rep_ctx:
              for t in range(n_tiles):
                # --- load 512 rows in one DMA: [128, 4, 784] ---
                x_sb = xin.tile([SUB, NSUB, D0], f32, tag="x")
                if in_dma_eng == "alt":
                    (nc.sync if t % 2 == 0 else nc.scalar).dma_start(
                        out=x_sb, in_=x_r[t])
                else:
                    getattr(nc, in_dma_eng).dma_start(out=x_sb, in_=x_r[t])

                # --- PE-transpose to feature-major: 7 chunks of [112, 512] ---
                xt_sb = []
                for c in range(NKC):
                    tp = psT.tile([KCH, TILE], f32, tag="psT")
                    for s in range(NSUB):
                        if r_xpose:
                            nc.tensor.transpose(
                                out=tp[:, s * SUB:(s + 1) * SUB]
                                    .bitcast(mybir.dt.float32r),
                                in_=x_sb[:, s, c * KCH:(c + 1) * KCH]
                                    .bitcast(mybir.dt.float32r),
                                identity=id_r,
                            )
                        else:
                            nc.tensor.transpose(
                                out=tp[:, s * SUB:(s + 1) * SUB],
                                in_=x_sb[:, s, c * KCH:(c + 1) * KCH],
                                identity=id_sb,
                            )
                    xt = xtp.tile([KCH, TILE], mm_dt, tag="xt")
                    if xt_on_act:
                        nc.scalar.activation(out=xt, in_=tp, func=COPY)
                    else:
                        nc.vector.tensor_copy(xt, tp)
                    xt_sb.append(xt)

                # --- L1: h1 = lrelu(W1 @ xT)  [128, 512] ---
                h1_ps = psMM.tile([D1, TILE], f32, tag="mm")
                for c in range(NKC):
                    nc.tensor.matmul(h1_ps, lhsT=w1t_sb[:, c, :], rhs=xt_sb[c],
                                     start=(c == 0), stop=(c == NKC - 1))
                h1_sb = acts.tile([D1, TILE], mm_dt, tag="h1")
                nc.scalar.activation(out=h1_sb, in_=h1_ps, func=LRELU, alpha=0.01)

                # --- L2: h2 = lrelu(W2 @ h1)  [64, 512] ---
                h2_ps = psMM.tile([D2, TILE], f32, tag="mm")
                nc.tensor.matmul(h2_ps, lhsT=w2t_sb, rhs=h1_sb,
                                 start=True, stop=True)
                h2_sb = acts.tile([D2, TILE], mm_dt, tag="h2")
                nc.scalar.activation(out=h2_sb, in_=h2_ps, func=LRELU, alpha=0.01)

                # --- L3 folded: g3 = lrelu((d3 @ W3) @ h2)  [64, 512] ---
                g3_ps = psMM.tile([D2, TILE], f32, tag="mm")
                nc.tensor.matmul(g3_ps, lhsT=m3t_sb, rhs=h2_sb,
                                 start=True, stop=True)
                g3_sb = acts.tile([D2, TILE], mm_dt, tag="g3")
                nc.scalar.activation(out=g3_sb, in_=g3_ps, func=LRELU, alpha=0.01)

                # --- L4: g2 = lrelu(d2 @ g3)  [128, 512] ---
                g2_ps = psMM.tile([D1, TILE], f32, tag="mm")
                nc.tensor.matmul(g2_ps, lhsT=d2t_sb, rhs=g3_sb,
                                 start=True, stop=True)
                g2_sb = acts.tile([D1, TILE], last_dt, tag="g2")
                nc.scalar.activation(out=g2_sb, in_=g2_ps, func=LRELU, alpha=0.01)

                # --- L5: out = g2.T @ d1.T, batch-major via stationary swap.
                # Two matmuls into one 2-bank PSUM tile ([:, :392] in bank 0,
                # [:, 512:904] in bank 1), one strided ACT copy out. ---
                o_sb = outp.tile([SUB, NSUB, D0], out_dt, tag="o")
                if no_l5:
                    nc.vector.tensor_copy(o_sb, x_sb)
                for s in range(NSUB) if not no_l5 else []:
                    g2c = g2_sb[:, s * SUB:(s + 1) * SUB]
                    po = psO.tile([SUB, 1024], f32, tag="po")
                    nc.tensor.matmul(po[:, :HALF], lhsT=g2c, rhs=d1t_sb[:, :HALF],
                                     start=True, stop=True)
                    nc.tensor.matmul(po[:, 512:512 + HALF], lhsT=g2c,
                                     rhs=d1t_sb[:, HALF:], start=True, stop=True)
                    po_v = po.rearrange("p (b r) -> p b r", b=2)[:, :, :HALF]
                    o_v = o_sb[:, s, :].rearrange("p (b r) -> p b r", b=2)
                    if split_ocopy and s % 2 == 1:
                        nc.vector.tensor_copy(o_v, po_v)
                    else:
                        nc.scalar.activation(out=o_v, in_=po_v, func=COPY)
                getattr(nc, out_dma_eng).dma_start(out=out_r[t], in_=o_sb)

    nc.finalize()
    return nc


def _host_weights(W1, W2, W3):
    def pinv(W):
        u, s, vh = np.linalg.svd(W.astype(np.float64), full_matrices=False)
        return (vh.T * (1.0 / s)) @ u.T

    d1, d2, d3 = pinv(W1), pinv(W2), pinv(W3)
    f = np.float32
    return {
        "w1t": np.ascontiguousarray(W1.T, dtype=f),
        "w2t": np.ascontiguousarray(W2.T, dtype=f),
        "m3t": np.ascontiguousarray((d3 @ W3.astype(np.float64)).T, dtype=f),
        "d2t": np.ascontiguousarray(d2.T, dtype=f),
        "d1t": np.ascontiguousarray(d1.T, dtype=f),
        "ident": np.eye(SUB, dtype=f),
    }


_NC_CACHE = {}


def _get_nc(key=("float32r", "float32r")):
    if key not in _NC_CACHE:
        _NC_CACHE[key] = _build_nc(B_LOC, key[0], key[1])
    return _NC_CACHE[key]


def kernel(x, W1, W2, W3):
    from concourse.bass_utils import run_bass_kernel_spmd

    x = np.ascontiguousarray(x, dtype=np.float32)
    w = _host_weights(np.asarray(W1), np.asarray(W2), np.asarray(W3))
    nc = _get_nc()
    in_maps = [
        {"x": x[i * B_LOC:(i + 1) * B_LOC], **w} for i in range(N_CORES)
    ]
    res = run_bass_kernel_spmd(nc, in_maps, core_ids=list(range(N_CORES)))
    return np.concatenate([res.results[i]["out"] for i in range(N_CORES)], axis=0)
